# revision 1
# baseline (speedup 1.0000x reference)
"""3x3 MedianBlur (zero-padded) for (8, 3, 1024, 1024) fp32 on 8 trn2 NeuronCores.

v2: bf16 compute path. The DVE runs tensor_tensor at 2x for packed 2-byte
dtypes (0.52 ns/elem vs 1.04 for fp32), and bf16 keeps the median exact to
~2^-8 relative (selection network only -- no arithmetic), far inside the 2e-2
gate. bf16 denormal range starts at 1e-38 so randn values never flush (fp16
would flush |x|<6e-5 and blow the max-rel-err metric).

  - Pure data parallel: batch element i -> core i.
  - Per core: 8 row-bands of 128 rows; rows live in partitions, (channel, col)
    in the free dim ([128, 3, 1026] tiles). Vertical window alignment comes
    free from DMA: each band is loaded 3x from HBM at row offsets -1/0/+1
    (fp32), then the ACT engine converts each to bf16 (ACT is otherwise idle;
    DVE is the bottleneck). Pool does the zero-pad memsets; PE idle.
  - Exact separable median-of-9: 18 bf16 min/max tensor_tensor ops per band
    on the DVE, all full-width, with in-place col-aligned tile reuse:
      stage V (6): m1,M1 = minmax(xm,x0); lo = min(m1,xp); t2 = min(M1,xp);
                   me = max(m1,t2) [->m1]; hi = max(M1,xp) [->M1]
      stage H (12): A = max3(lo), C = min3(hi), B = med3(me) via pair
                   partials, out = med3(A,B,C)
  - ACT converts the final bf16 band back to fp32 (split by channel so the
    store overlaps the next band's compute); stores on the scalar queue.
  - Software pipelining by emission order: band t+1's loads+converts are
    emitted before band t's compute so the in-order ACT stream never parks
    conversions behind an output conversion. Band 0 is emitted per-channel
    to cut the pipeline fill; the last band is split at col 832 so the final
    store chain drains behind a small compute piece.
  - All loads ride the sync queue (scalar-queue dma_starts hold the ACT SEQ
    through the shared HWDGE generator and delay conversions); band 0's first
    channel is col-halved so the DVE starts ~5.5us in; the last band splits
    at col 832 with per-channel stores, and its final piece's last op writes
    f32 directly (one op at fp32 rate buys the drain an ACT-conversion hop).
  - Band 0's first piece converts on the (then-idle) DVE, skipping the
    ACT->DVE semaphore hop in the fill chain.
  - f32 staging tiles live in a single-buffered pool (dead after the
    bf16 conversion; single-buffering trims semaphore traffic and SBUF).
  - Cost-model makespan: 253.1 us (DVE busy ~243 us / 96%; fp32 DVE floor
    for this 18-op network is ~470 us -- the bf16 2x mode is the win. Pool
    cannot run tensor_tensor in this walrus build, ACT has no elementwise
    2-tensor op, and PE is linear-only, so the DVE does all 18 ops).

The walrus build accepts at most 1 inline sync wait per instruction
(2 on EventSemaphore); Tile emits more, so _legalize_waits() spills excess
waits onto same-engine NoOps placed immediately before the instruction.
"""
import sys

sys.path.insert(0, "/opt/trn_rl_repo")

import numpy as np

import concourse.bass as bass
import concourse.mybir as mybir
from concourse.bass_utils import run_bass_kernel_spmd
from concourse.tile import TileContext


C, H, W = 3, 1024, 1024
P = 128
NT = H // P
SP = W + 2      # padded width: tile col c <-> DRAM col c-1
F32 = mybir.dt.float32
BF16 = mybir.dt.bfloat16
MIN = mybir.AluOpType.min
MAX = mybir.AluOpType.max


def _legalize_waits(nc):
    """Split sync_info.on_wait lists that exceed this walrus's per-instruction
    capacity (1; 2 for EventSemaphore) onto preceding same-engine NoOps."""
    for f in nc.m.functions:
        for bb in f.blocks:
            new_insts = []
            for ins in bb.instructions:
                si = ins.sync_info
                cap = 2 if ins.opcode == "EventSemaphore" else 1
                if si is not None and len(si.on_wait) > cap:
                    waits = list(si.on_wait)
                    extra, keep = waits[:-cap], waits[-cap:]
                    for w in extra:
                        nop = mybir.InstNoOp(
                            name=nc.get_next_instruction_name(),
                            ins=[],
                            outs=[],
                            engine=ins.engine,
                        )
                        nop.sync_info = mybir.SyncInfo(on_wait=[w], on_update=[])
                        new_insts.append(nop)
                    ins.sync_info = mybir.SyncInfo(
                        on_wait=keep, on_update=list(si.on_update)
                    )
                new_insts.append(ins)
            bb.instructions = new_insts


def build(bufs=2):
    nc = bass.Bass()
    x = nc.dram_tensor("x", [C, H, W], F32, kind="ExternalInput")
    y = nc.dram_tensor("y", [C, H, W], F32, kind="ExternalOutput")
    tt = nc.vector.tensor_tensor

    with TileContext(nc) as tc:
        with (
            tc.tile_pool(name="pool", bufs=bufs) as pool,
            tc.tile_pool(name="fpool", bufs=1) as fpool,
        ):
            band = {}  # t -> dict of tiles

            def emit_load(t, chunks):
                """Loads (f32) + bf16 conversion + pad memsets for band t.

                chunks: list of (c0, c1, w0, w1) channel/DRAM-col pieces."""
                r0 = t * P
                xmf = fpool.tile([P, C, SP], F32, tag="xmf")
                x0f = fpool.tile([P, C, SP], F32, tag="x0f")
                xpf = fpool.tile([P, C, SP], F32, tag="xpf")
                xm = pool.tile([P, C, SP], BF16, tag="xm")
                x0 = pool.tile([P, C, SP], BF16, tag="x0")
                xp = pool.tile([P, C, SP], BF16, tag="xp")
                band[t] = {"xm": xm, "x0": x0, "xp": xp}
                # zero pad: tile cols 0 and 1025 of each bf16 tile
                for z in (xm, x0, xp):
                    e = z[:].copy()
                    e.ap = e.ap.__class__(
                        [tuple(e.ap[0]), (SP, C), (W + 1, 2)]
                    )
                    nc.gpsimd.memset(e, 0.0)
                for c0, c1, w0, w1 in chunks:
                    ta, tb = w0 + 1, w1 + 1  # tile col range of this piece
                    # pad rows: memset the f32 staging pad row before
                    # conversion (compute APs must start at partition 0, so
                    # [0:1] is legal but [1:P] is not; conversions cover [0:P])
                    if t == 0:
                        nc.gpsimd.memset(xmf[0:1, c0:c1, ta:tb], 0.0)
                    if t == NT - 1:
                        # base must be 32-aligned; DMA rewrites rows 96..126
                        nc.gpsimd.memset(xpf[96:P, c0:c1, ta:tb], 0.0)
                    # xm first: V's first two ops need only xm+x0, so their
                    # conversions gate DVE start. xm: rows r0-1..r0+126
                    if t == 0:
                        nc.sync.dma_start(
                            xmf[1:P, c0:c1, ta:tb],
                            x[c0:c1, 0 : P - 1, w0:w1].rearrange("c r w -> r c w"),
                        )
                    else:
                        nc.sync.dma_start(
                            xmf[:, c0:c1, ta:tb],
                            x[c0:c1, r0 - 1 : r0 + P - 1, w0:w1].rearrange(
                                "c r w -> r c w"
                            ),
                        )
                    # x0: rows r0..r0+127 (sync queue: keep the ACT SEQ
                    # free of HWDGE-serialized dma_starts so cvts run early)
                    nc.sync.dma_start(
                        x0f[:, c0:c1, ta:tb],
                        x[c0:c1, r0 : r0 + P, w0:w1].rearrange("c r w -> r c w"),
                    )
                    # xp: rows r0+1..r0+128
                    if t == NT - 1:
                        nc.sync.dma_start(
                            xpf[0 : P - 1, c0:c1, ta:tb],
                            x[c0:c1, r0 + 1 : r0 + P, w0:w1].rearrange(
                                "c r w -> r c w"
                            ),
                        )
                    else:
                        nc.sync.dma_start(
                            xpf[:, c0:c1, ta:tb],
                            x[c0:c1, r0 + 1 : r0 + P + 1, w0:w1].rearrange(
                                "c r w -> r c w"
                            ),
                        )
                    # bf16 conversions on ACT (full partition range; pad cols
                    # handled by the bf16 memsets above). Exception: the very
                    # first piece converts on the idle DVE -- it skips the
                    # ACT->DVE semaphore hop that gates the pipeline fill.
                    cvt = (
                        nc.vector.tensor_copy
                        if (t == 0 and (c0, c1, w0, w1) == chunks[0])
                        else nc.scalar.copy
                    )
                    cvt(xm[:, c0:c1, ta:tb], xmf[:, c0:c1, ta:tb])
                    cvt(x0[:, c0:c1, ta:tb], x0f[:, c0:c1, ta:tb])
                    cvt(xp[:, c0:c1, ta:tb], xpf[:, c0:c1, ta:tb])

            def emit_compute(t, chunks, store_chunks=None, fuse_last=False):
                r0 = t * P
                d = band[t]
                xm, x0, xp = d["xm"], d["x0"], d["xp"]
                m1 = pool.tile([P, C, SP], BF16, tag="m1")
                M1 = pool.tile([P, C, SP], BF16, tag="M1")
                lo = pool.tile([P, C, SP], BF16, tag="lo")
                t2 = pool.tile([P, C, SP], BF16, tag="t2")
                outf = pool.tile([P, C, W], F32, tag="outf")
                for c0, c1, w0, w1 in chunks:
                    # stage V over tile cols [w0, w1+2); adjacent chunks
                    # recompute the 2 overlap cols (same values, benign)
                    va, vb = w0, w1 + 2
                    tt(m1[:, c0:c1, va:vb], xm[:, c0:c1, va:vb], x0[:, c0:c1, va:vb], MIN)
                    tt(M1[:, c0:c1, va:vb], xm[:, c0:c1, va:vb], x0[:, c0:c1, va:vb], MAX)
                    tt(lo[:, c0:c1, va:vb], m1[:, c0:c1, va:vb], xp[:, c0:c1, va:vb], MIN)
                    tt(t2[:, c0:c1, va:vb], M1[:, c0:c1, va:vb], xp[:, c0:c1, va:vb], MIN)
                    tt(m1[:, c0:c1, va:vb], m1[:, c0:c1, va:vb], t2[:, c0:c1, va:vb], MAX)  # me
                    tt(M1[:, c0:c1, va:vb], M1[:, c0:c1, va:vb], xp[:, c0:c1, va:vb], MAX)  # hi
                    me, hi = m1, M1
                    # stage H over out cols [w0, w1); reuse dead tiles:
                    # pa->xm, pc->xp, q->x0, p->t2 (names = storage)
                    # partials confined to [w0, w1): the w1'th value is never
                    # used, and writing it would clobber the aliased x tiles
                    # at the next col-chunk's V halo
                    h1 = w1 + 1
                    tt(xm[:, c0:c1, w0:w1], lo[:, c0:c1, w0:w1], lo[:, c0:c1, w0 + 1 : h1], MAX)  # pa
                    tt(xp[:, c0:c1, w0:w1], hi[:, c0:c1, w0:w1], hi[:, c0:c1, w0 + 1 : h1], MIN)  # pc
                    tt(x0[:, c0:c1, w0:w1], me[:, c0:c1, w0:w1], me[:, c0:c1, w0 + 1 : h1], MAX)  # q
                    tt(t2[:, c0:c1, w0:w1], me[:, c0:c1, w0:w1], me[:, c0:c1, w0 + 1 : h1], MIN)  # p
                    tt(xm[:, c0:c1, w0:w1], xm[:, c0:c1, w0:w1], lo[:, c0:c1, w0 + 2 : vb], MAX)  # A
                    tt(xp[:, c0:c1, w0:w1], xp[:, c0:c1, w0:w1], hi[:, c0:c1, w0 + 2 : vb], MIN)  # Cc
                    tt(x0[:, c0:c1, w0:w1], x0[:, c0:c1, w0:w1], me[:, c0:c1, w0 + 2 : vb], MIN)  # b1
                    tt(x0[:, c0:c1, w0:w1], t2[:, c0:c1, w0:w1], x0[:, c0:c1, w0:w1], MAX)  # B
                    A, B, Cc = xm, x0, xp
                    tt(lo[:, c0:c1, w0:w1], A[:, c0:c1, w0:w1], B[:, c0:c1, w0:w1], MIN)  # m2
                    tt(xm[:, c0:c1, w0:w1], A[:, c0:c1, w0:w1], B[:, c0:c1, w0:w1], MAX)  # M2
                    tt(xp[:, c0:c1, w0:w1], xm[:, c0:c1, w0:w1], Cc[:, c0:c1, w0:w1], MIN)  # t3
                    if fuse_last and (c0, c1, w0, w1) == chunks[-1]:
                        # final piece: write f32 directly (fp32 rate on this
                        # one op) so the drain skips the ACT conversion hop
                        tt(outf[:, c0:c1, w0:w1], xp[:, c0:c1, w0:w1], lo[:, c0:c1, w0:w1], MAX)
                    else:
                        tt(lo[:, c0:c1, w0:w1], xp[:, c0:c1, w0:w1], lo[:, c0:c1, w0:w1], MAX)  # out
                # output conversion + store, split for tail overlap
                if len(chunks) == 1:
                    oc = ((0, 2, 0, W), (2, 3, 0, W))
                elif store_chunks is not None:
                    oc = store_chunks
                else:
                    oc = chunks
                for c0, c1, w0, w1 in oc:
                    if not (fuse_last and (c0, c1, w0, w1) == oc[-1]):
                        nc.scalar.copy(outf[:, c0:c1, w0:w1], lo[:, c0:c1, w0:w1])
                    nc.scalar.dma_start(
                        y[c0:c1, r0 : r0 + P, w0:w1].rearrange("c r w -> r c w"),
                        outf[:, c0:c1, w0:w1],
                    )

            full = [(0, C, 0, W)]
            # band 0: small first piece so DVE starts ~4us in (load split at
            # 260 vs compute split at 258: piece 1's V halo stays in load 1)
            load0 = [(0, 1, 0, 418), (0, 1, 418, W), (1, 3, 0, W)]
            comp0 = [(0, 1, 0, 416), (0, 1, 416, W), (1, 3, 0, W)]
            # last band: small last piece for a short drain (768 balances the
            # middle piece's store chain against the last piece's compute)
            tailc = [(0, 3, 0, 832), (0, 3, 832, W)]
            tail_store = [(0, 2, 0, 832), (2, 3, 0, 832), (0, 3, 832, W)]
            emit_load(0, load0)
            emit_load(1, full)
            for t in range(NT):
                if t == 0:
                    chunks = comp0
                elif t == NT - 1:
                    chunks = tailc
                else:
                    chunks = full
                emit_compute(t, chunks,
                             tail_store if t == NT - 1 else None,
                             fuse_last=(t == NT - 1))
                if t + 2 < NT:
                    emit_load(t + 2, full)

    _legalize_waits(nc)
    return nc


_NC = None


def kernel(input):
    global _NC
    if _NC is None:
        _NC = build()
    input = np.asarray(input, dtype=np.float32)
    in_maps = [{"x": np.ascontiguousarray(input[i])} for i in range(input.shape[0])]
    res = run_bass_kernel_spmd(_NC, in_maps, core_ids=list(range(len(in_maps))))
    return np.stack([r["y"] for r in res.results], axis=0)



# revision 16
# speedup vs baseline: 1.0030x; 1.0030x over previous
"""3x3 MedianBlur (zero-padded) for (8, 3, 1024, 1024) fp32 on 8 trn2 NeuronCores.

v2: bf16 compute path. The DVE runs tensor_tensor at 2x for packed 2-byte
dtypes (0.52 ns/elem vs 1.04 for fp32), and bf16 keeps the median exact to
~2^-8 relative (selection network only -- no arithmetic), far inside the 2e-2
gate. bf16 denormal range starts at 1e-38 so randn values never flush (fp16
would flush |x|<6e-5 and blow the max-rel-err metric).

  - Pure data parallel: batch element i -> core i.
  - Per core: 8 row-bands of 128 rows; rows live in partitions, (channel, col)
    in the free dim ([128, 3, 1026] tiles). Vertical window alignment comes
    free from DMA: each band is loaded 3x from HBM at row offsets -1/0/+1
    (fp32), then the ACT engine converts each to bf16 (ACT is otherwise idle;
    DVE is the bottleneck). Pool does the zero-pad memsets; PE idle.
  - Exact separable median-of-9: 18 bf16 min/max tensor_tensor ops per band
    on the DVE, all full-width, with in-place col-aligned tile reuse:
      stage V (6): m1,M1 = minmax(xm,x0); lo = min(m1,xp); t2 = min(M1,xp);
                   me = max(m1,t2) [->m1]; hi = max(M1,xp) [->M1]
      stage H (12): A = max3(lo), C = min3(hi), B = med3(me) via pair
                   partials, out = med3(A,B,C)
  - ACT converts the final bf16 band back to fp32 (split by channel so the
    store overlaps the next band's compute); stores on the scalar queue.
  - Software pipelining by emission order: band t+1's loads+converts are
    emitted before band t's compute so the in-order ACT stream never parks
    conversions behind an output conversion. Band 0 is emitted per-channel
    to cut the pipeline fill; the last band is split at col 832 so the final
    store chain drains behind a small compute piece.
  - All loads ride the sync queue (scalar-queue dma_starts hold the ACT SEQ
    through the shared HWDGE generator and delay conversions); band 0's first
    channel is col-halved so the DVE starts ~5.5us in; the last band splits
    at col 832 with per-channel stores, and its final piece's last op writes
    f32 directly (one op at fp32 rate buys the drain an ACT-conversion hop).
  - Band 0's first piece converts on the (then-idle) DVE, skipping the
    ACT->DVE semaphore hop in the fill chain.
  - f32 staging tiles live in a single-buffered pool (dead after the
    bf16 conversion; single-buffering trims semaphore traffic and SBUF).
  - Cost-model makespan: 253.1 us (DVE busy ~243 us / 96%; fp32 DVE floor
    for this 18-op network is ~470 us -- the bf16 2x mode is the win. Pool
    cannot run tensor_tensor in this walrus build, ACT has no elementwise
    2-tensor op, and PE is linear-only, so the DVE does all 18 ops).

The walrus build accepts at most 1 inline sync wait per instruction
(2 on EventSemaphore); Tile emits more, so _legalize_waits() spills excess
waits onto same-engine NoOps placed immediately before the instruction.
"""
import sys

sys.path.insert(0, "/opt/trn_rl_repo")

import numpy as np

import concourse.bass as bass
import concourse.mybir as mybir
from concourse.bass_utils import run_bass_kernel_spmd
from concourse.tile import TileContext


C, H, W = 3, 1024, 1024
P = 128
NT = H // P
SP = W + 2      # padded width: tile col c <-> DRAM col c-1
F32 = mybir.dt.float32
BF16 = mybir.dt.bfloat16
MIN = mybir.AluOpType.min
MAX = mybir.AluOpType.max


def _legalize_waits(nc):
    """Split sync_info.on_wait lists that exceed this walrus's per-instruction
    capacity (1; 2 for EventSemaphore) onto preceding same-engine NoOps."""
    for f in nc.m.functions:
        for bb in f.blocks:
            new_insts = []
            for ins in bb.instructions:
                si = ins.sync_info
                cap = 2 if ins.opcode == "EventSemaphore" else 1
                if si is not None and len(si.on_wait) > cap:
                    waits = list(si.on_wait)
                    extra, keep = waits[:-cap], waits[-cap:]
                    for w in extra:
                        nop = mybir.InstNoOp(
                            name=nc.get_next_instruction_name(),
                            ins=[],
                            outs=[],
                            engine=ins.engine,
                        )
                        nop.sync_info = mybir.SyncInfo(on_wait=[w], on_update=[])
                        new_insts.append(nop)
                    ins.sync_info = mybir.SyncInfo(
                        on_wait=keep, on_update=list(si.on_update)
                    )
                new_insts.append(ins)
            bb.instructions = new_insts


def build(bufs=2):
    nc = bass.Bass()
    x = nc.dram_tensor("x", [C, H, W], F32, kind="ExternalInput")
    y = nc.dram_tensor("y", [C, H, W], F32, kind="ExternalOutput")
    tt = nc.vector.tensor_tensor

    with TileContext(nc) as tc:
        with (
            tc.tile_pool(name="pool", bufs=bufs) as pool,
            tc.tile_pool(name="fpool", bufs=1) as fpool,
            tc.tile_pool(name="opool", bufs=3) as opool,
        ):
            band = {}  # t -> dict of tiles

            def emit_load(t, chunks):
                """Loads (f32) + bf16 conversion + pad memsets for band t.

                chunks: list of (c0, c1, w0, w1) channel/DRAM-col pieces."""
                r0 = t * P
                xmf = fpool.tile([P, C, SP], F32, tag="xmf")
                x0f = fpool.tile([P, C, SP], F32, tag="x0f")
                xpf = fpool.tile([P, C, SP], F32, tag="xpf")
                xm = pool.tile([P, C, SP], BF16, tag="xm")
                x0 = pool.tile([P, C, SP], BF16, tag="x0")
                xp = pool.tile([P, C, SP], BF16, tag="xp")
                band[t] = {"xm": xm, "x0": x0, "xp": xp}
                # zero pad: tile cols 0 and 1025 of each bf16 tile
                for z in (xm, x0, xp):
                    e = z[:].copy()
                    e.ap = e.ap.__class__(
                        [tuple(e.ap[0]), (SP, C), (W + 1, 2)]
                    )
                    nc.gpsimd.memset(e, 0.0)
                for c0, c1, w0, w1 in chunks:
                    ta, tb = w0 + 1, w1 + 1  # tile col range of this piece
                    # pad rows: memset the f32 staging pad row before
                    # conversion (compute APs must start at partition 0, so
                    # [0:1] is legal but [1:P] is not; conversions cover [0:P])
                    if t == 0:
                        nc.gpsimd.memset(xmf[0:1, c0:c1, ta:tb], 0.0)
                    if t == NT - 1:
                        # base must be 32-aligned; DMA rewrites rows 96..126
                        nc.gpsimd.memset(xpf[96:P, c0:c1, ta:tb], 0.0)
                    # xm first: V's first two ops need only xm+x0, so their
                    # conversions gate DVE start. xm: rows r0-1..r0+126
                    if t == 0:
                        nc.sync.dma_start(
                            xmf[1:P, c0:c1, ta:tb],
                            x[c0:c1, 0 : P - 1, w0:w1].rearrange("c r w -> r c w"),
                        )
                    else:
                        nc.sync.dma_start(
                            xmf[:, c0:c1, ta:tb],
                            x[c0:c1, r0 - 1 : r0 + P - 1, w0:w1].rearrange(
                                "c r w -> r c w"
                            ),
                        )
                    # x0: rows r0..r0+127 (sync queue: keep the ACT SEQ
                    # free of HWDGE-serialized dma_starts so cvts run early)
                    nc.sync.dma_start(
                        x0f[:, c0:c1, ta:tb],
                        x[c0:c1, r0 : r0 + P, w0:w1].rearrange("c r w -> r c w"),
                    )
                    # xp: rows r0+1..r0+128
                    if t == NT - 1:
                        nc.sync.dma_start(
                            xpf[0 : P - 1, c0:c1, ta:tb],
                            x[c0:c1, r0 + 1 : r0 + P, w0:w1].rearrange(
                                "c r w -> r c w"
                            ),
                        )
                    else:
                        nc.sync.dma_start(
                            xpf[:, c0:c1, ta:tb],
                            x[c0:c1, r0 + 1 : r0 + P + 1, w0:w1].rearrange(
                                "c r w -> r c w"
                            ),
                        )
                    # bf16 conversions on ACT (full partition range; pad cols
                    # handled by the bf16 memsets above). Exception: the very
                    # first piece converts on the idle DVE -- it skips the
                    # ACT->DVE semaphore hop that gates the pipeline fill.
                    cvt = (
                        nc.vector.tensor_copy
                        if (t == 0 and (c0, c1, w0, w1) == chunks[0])
                        else nc.scalar.copy
                    )
                    cvt(xm[:, c0:c1, ta:tb], xmf[:, c0:c1, ta:tb])
                    cvt(x0[:, c0:c1, ta:tb], x0f[:, c0:c1, ta:tb])
                    cvt(xp[:, c0:c1, ta:tb], xpf[:, c0:c1, ta:tb])

            def emit_compute(t, chunks, store_chunks=None, fuse_last=False):
                r0 = t * P
                d = band[t]
                xm, x0, xp = d["xm"], d["x0"], d["xp"]
                m1 = pool.tile([P, C, SP], BF16, tag="m1")
                M1 = pool.tile([P, C, SP], BF16, tag="M1")
                lo = pool.tile([P, C, SP], BF16, tag="lo")
                t2 = pool.tile([P, C, SP], BF16, tag="t2")
                outf = opool.tile([P, C, W], F32, tag="outf")
                if fuse_last:
                    outfB = opool.tile([P, C, W - TB], F32, tag="outfB", name="outfB")
                else:
                    outfB = None
                for c0, c1, w0, w1 in chunks:
                    # stage V over tile cols [w0, w1+2); adjacent chunks
                    # recompute the 2 overlap cols (same values, benign)
                    va, vb = w0, w1 + 2
                    tt(m1[:, c0:c1, va:vb], xm[:, c0:c1, va:vb], x0[:, c0:c1, va:vb], MIN)
                    tt(M1[:, c0:c1, va:vb], xm[:, c0:c1, va:vb], x0[:, c0:c1, va:vb], MAX)
                    tt(lo[:, c0:c1, va:vb], m1[:, c0:c1, va:vb], xp[:, c0:c1, va:vb], MIN)
                    tt(t2[:, c0:c1, va:vb], M1[:, c0:c1, va:vb], xp[:, c0:c1, va:vb], MIN)
                    tt(m1[:, c0:c1, va:vb], m1[:, c0:c1, va:vb], t2[:, c0:c1, va:vb], MAX)  # me
                    tt(M1[:, c0:c1, va:vb], M1[:, c0:c1, va:vb], xp[:, c0:c1, va:vb], MAX)  # hi
                    me, hi = m1, M1
                    # stage H over out cols [w0, w1); reuse dead tiles:
                    # pa->xm, pc->xp, q->x0, p->t2 (names = storage)
                    # partials confined to [w0, w1): the w1'th value is never
                    # used, and writing it would clobber the aliased x tiles
                    # at the next col-chunk's V halo
                    h1 = w1 + 1
                    tt(xm[:, c0:c1, w0:w1], lo[:, c0:c1, w0:w1], lo[:, c0:c1, w0 + 1 : h1], MAX)  # pa
                    tt(xp[:, c0:c1, w0:w1], hi[:, c0:c1, w0:w1], hi[:, c0:c1, w0 + 1 : h1], MIN)  # pc
                    tt(x0[:, c0:c1, w0:w1], me[:, c0:c1, w0:w1], me[:, c0:c1, w0 + 1 : h1], MAX)  # q
                    tt(t2[:, c0:c1, w0:w1], me[:, c0:c1, w0:w1], me[:, c0:c1, w0 + 1 : h1], MIN)  # p
                    tt(xm[:, c0:c1, w0:w1], xm[:, c0:c1, w0:w1], lo[:, c0:c1, w0 + 2 : vb], MAX)  # A
                    tt(xp[:, c0:c1, w0:w1], xp[:, c0:c1, w0:w1], hi[:, c0:c1, w0 + 2 : vb], MIN)  # Cc
                    tt(x0[:, c0:c1, w0:w1], x0[:, c0:c1, w0:w1], me[:, c0:c1, w0 + 2 : vb], MIN)  # b1
                    tt(x0[:, c0:c1, w0:w1], t2[:, c0:c1, w0:w1], x0[:, c0:c1, w0:w1], MAX)  # B
                    A, B, Cc = xm, x0, xp
                    tt(lo[:, c0:c1, w0:w1], A[:, c0:c1, w0:w1], B[:, c0:c1, w0:w1], MIN)  # m2
                    tt(xm[:, c0:c1, w0:w1], A[:, c0:c1, w0:w1], B[:, c0:c1, w0:w1], MAX)  # M2
                    tt(xp[:, c0:c1, w0:w1], xm[:, c0:c1, w0:w1], Cc[:, c0:c1, w0:w1], MIN)  # t3
                    if fuse_last and (c0, c1, w0, w1) == chunks[-1]:
                        # final piece: write f32 directly (fp32 rate on this
                        # one op) into its own tile so earlier stores don't
                        # pick up a false dep on the last DVE op
                        tt(outfB[:, c0:c1, 0 : w1 - w0], xp[:, c0:c1, w0:w1], lo[:, c0:c1, w0:w1], MAX)
                    else:
                        tt(lo[:, c0:c1, w0:w1], xp[:, c0:c1, w0:w1], lo[:, c0:c1, w0:w1], MAX)  # out
                # output conversion + store, split for tail overlap
                if len(chunks) == 1:
                    oc = ((0, 2, 0, W), (2, 3, 0, W))
                elif store_chunks is not None:
                    oc = store_chunks
                else:
                    oc = chunks
                for ch in oc:
                    c0, c1, w0, w1 = ch[:4]
                    q = ch[4] if len(ch) > 4 else "a"
                    ce = ch[5] if len(ch) > 5 else "a"
                    if not (fuse_last and ch == oc[-1]):
                        cvt = nc.scalar.copy if ce == "a" else nc.vector.tensor_copy
                        cvt(outf[:, c0:c1, w0:w1], lo[:, c0:c1, w0:w1])
                    eng = {"a": nc.scalar, "s": nc.sync, "p": nc.gpsimd}[q]
                    if fuse_last and ch == oc[-1]:
                        src = outfB[:, c0:c1, 0 : w1 - w0]
                    else:
                        src = outf[:, c0:c1, w0:w1]
                    eng.dma_start(
                        y[c0:c1, r0 : r0 + P, w0:w1].rearrange("c r w -> r c w"),
                        src,
                    )

            full = [(0, C, 0, W)]
            # band 0: small first piece so DVE starts early (load split 2 cols
            # past the compute split so piece 1's V halo stays in load 1)
            load0 = [(0, 1, 0, 418), (0, 1, 418, W), (1, 3, 0, W)]
            comp0 = [(0, 1, 0, 416), (0, 1, 416, W), (1, 3, 0, W)]
            # last band: small last piece for a short drain; stores fan out
            # across the ACT and SP queues so the drain chains run in parallel
            TB = 928  # tail split col
            tailc = [(0, 3, 0, TB), (0, 3, TB, W)]
            tail_store = [
                (0, 2, 0, TB, "a", "a"),
                (2, 3, 0, TB, "s", "v"),
                (0, 3, TB, W, "p", "a"),
            ]
            emit_load(0, load0)
            emit_load(1, full)
            for t in range(NT):
                if t == 0:
                    chunks = comp0
                elif t == NT - 1:
                    chunks = tailc
                else:
                    chunks = full
                emit_compute(t, chunks,
                             tail_store if t == NT - 1 else None,
                             fuse_last=(t == NT - 1))
                if t + 2 < NT:
                    emit_load(t + 2, full)

    _legalize_waits(nc)
    return nc


_NC = None


def kernel(input):
    global _NC
    if _NC is None:
        _NC = build()
    input = np.asarray(input, dtype=np.float32)
    in_maps = [{"x": np.ascontiguousarray(input[i])} for i in range(input.shape[0])]
    res = run_bass_kernel_spmd(_NC, in_maps, core_ids=list(range(len(in_maps))))
    return np.stack([r["y"] for r in res.results], axis=0)



# revision 19
# speedup vs baseline: 1.0045x; 1.0015x over previous
"""3x3 MedianBlur (zero-padded) for (8, 3, 1024, 1024) fp32 on 8 trn2 NeuronCores.

v2: bf16 compute path. The DVE runs tensor_tensor at 2x for packed 2-byte
dtypes (0.52 ns/elem vs 1.04 for fp32), and bf16 keeps the median exact to
~2^-8 relative (selection network only -- no arithmetic), far inside the 2e-2
gate. bf16 denormal range starts at 1e-38 so randn values never flush (fp16
would flush |x|<6e-5 and blow the max-rel-err metric).

  - Pure data parallel: batch element i -> core i.
  - Per core: 8 row-bands of 128 rows; rows live in partitions, (channel, col)
    in the free dim ([128, 3, 1026] tiles). Vertical window alignment comes
    free from DMA: each band is loaded 3x from HBM at row offsets -1/0/+1
    (fp32), then the ACT engine converts each to bf16 (ACT is otherwise idle;
    DVE is the bottleneck). Pool does the zero-pad memsets; PE idle.
  - Exact separable median-of-9: 18 bf16 min/max tensor_tensor ops per band
    on the DVE, all full-width, with in-place col-aligned tile reuse:
      stage V (6): m1,M1 = minmax(xm,x0); lo = min(m1,xp); t2 = min(M1,xp);
                   me = max(m1,t2) [->m1]; hi = max(M1,xp) [->M1]
      stage H (12): A = max3(lo), C = min3(hi), B = med3(me) via pair
                   partials, out = med3(A,B,C)
  - ACT converts the final bf16 band back to fp32 (split by channel so the
    store overlaps the next band's compute); stores on the scalar queue.
  - Software pipelining by emission order: band t+1's loads+converts are
    emitted before band t's compute so the in-order ACT stream never parks
    conversions behind an output conversion. Band 0 is emitted per-channel
    to cut the pipeline fill; the last band is split at col 832 so the final
    store chain drains behind a small compute piece.
  - All loads ride the sync queue (scalar-queue dma_starts hold the ACT SEQ
    through the shared HWDGE generator and delay conversions); band 0's first
    channel is col-halved so the DVE starts ~5.5us in; the last band splits
    at col 832 with per-channel stores, and its final piece's last op writes
    f32 directly (one op at fp32 rate buys the drain an ACT-conversion hop).
  - Band 0's first piece converts on the (then-idle) DVE, skipping the
    ACT->DVE semaphore hop in the fill chain.
  - f32 staging tiles live in a single-buffered pool (dead after the
    bf16 conversion; single-buffering trims semaphore traffic and SBUF).
  - Cost-model makespan: 253.1 us (DVE busy ~243 us / 96%; fp32 DVE floor
    for this 18-op network is ~470 us -- the bf16 2x mode is the win. Pool
    cannot run tensor_tensor in this walrus build, ACT has no elementwise
    2-tensor op, and PE is linear-only, so the DVE does all 18 ops).

The walrus build accepts at most 1 inline sync wait per instruction
(2 on EventSemaphore); Tile emits more, so _legalize_waits() spills excess
waits onto same-engine NoOps placed immediately before the instruction.
"""
import sys

sys.path.insert(0, "/opt/trn_rl_repo")

import numpy as np

import concourse.bass as bass
import concourse.mybir as mybir
from concourse.bass_utils import run_bass_kernel_spmd
from concourse.tile import TileContext


C, H, W = 3, 1024, 1024
P = 128
NT = H // P
SP = W + 2      # padded width: tile col c <-> DRAM col c-1
F32 = mybir.dt.float32
BF16 = mybir.dt.bfloat16
MIN = mybir.AluOpType.min
MAX = mybir.AluOpType.max


def _legalize_waits(nc):
    """Split sync_info.on_wait lists that exceed this walrus's per-instruction
    capacity (1; 2 for EventSemaphore) onto preceding same-engine NoOps."""
    for f in nc.m.functions:
        for bb in f.blocks:
            new_insts = []
            for ins in bb.instructions:
                si = ins.sync_info
                cap = 2 if ins.opcode == "EventSemaphore" else 1
                if si is not None and len(si.on_wait) > cap:
                    waits = list(si.on_wait)
                    extra, keep = waits[:-cap], waits[-cap:]
                    for w in extra:
                        nop = mybir.InstNoOp(
                            name=nc.get_next_instruction_name(),
                            ins=[],
                            outs=[],
                            engine=ins.engine,
                        )
                        nop.sync_info = mybir.SyncInfo(on_wait=[w], on_update=[])
                        new_insts.append(nop)
                    ins.sync_info = mybir.SyncInfo(
                        on_wait=keep, on_update=list(si.on_update)
                    )
                new_insts.append(ins)
            bb.instructions = new_insts


def build(bufs=2, s0=520):
    nc = bass.Bass()
    x = nc.dram_tensor("x", [C, H, W], F32, kind="ExternalInput")
    y = nc.dram_tensor("y", [C, H, W], F32, kind="ExternalOutput")
    tt = nc.vector.tensor_tensor

    with TileContext(nc) as tc:
        with (
            tc.tile_pool(name="pool", bufs=bufs) as pool,
            tc.tile_pool(name="fpool", bufs=1) as fpool,
            tc.tile_pool(name="opool", bufs=3) as opool,
        ):
            band = {}  # t -> dict of tiles

            def emit_load(t, chunks):
                """Loads (f32) + bf16 conversion + pad memsets for band t.

                chunks: list of (c0, c1, w0, w1) channel/DRAM-col pieces."""
                r0 = t * P
                xmf = fpool.tile([P, C, SP], F32, tag="xmf")
                x0f = fpool.tile([P, C, SP], F32, tag="x0f")
                xpf = fpool.tile([P, C, SP], F32, tag="xpf")
                xm = pool.tile([P, C, SP], BF16, tag="xm")
                x0 = pool.tile([P, C, SP], BF16, tag="x0")
                xp = pool.tile([P, C, SP], BF16, tag="xp")
                band[t] = {"xm": xm, "x0": x0, "xp": xp}
                # zero pad: tile cols 0 and 1025 of each bf16 tile
                for z in (xm, x0, xp):
                    e = z[:].copy()
                    e.ap = e.ap.__class__(
                        [tuple(e.ap[0]), (SP, C), (W + 1, 2)]
                    )
                    nc.gpsimd.memset(e, 0.0)
                for c0, c1, w0, w1 in chunks:
                    ta, tb = w0 + 1, w1 + 1  # tile col range of this piece
                    # pad rows: memset the f32 staging pad row before
                    # conversion (compute APs must start at partition 0, so
                    # [0:1] is legal but [1:P] is not; conversions cover [0:P])
                    if t == 0:
                        nc.gpsimd.memset(xmf[0:1, c0:c1, ta:tb], 0.0)
                    if t == NT - 1:
                        # base must be 32-aligned; DMA rewrites rows 96..126
                        nc.gpsimd.memset(xpf[96:P, c0:c1, ta:tb], 0.0)
                    # xm first: V's first two ops need only xm+x0, so their
                    # conversions gate DVE start. xm: rows r0-1..r0+126
                    if t == 0:
                        nc.sync.dma_start(
                            xmf[1:P, c0:c1, ta:tb],
                            x[c0:c1, 0 : P - 1, w0:w1].rearrange("c r w -> r c w"),
                        )
                    else:
                        nc.sync.dma_start(
                            xmf[:, c0:c1, ta:tb],
                            x[c0:c1, r0 - 1 : r0 + P - 1, w0:w1].rearrange(
                                "c r w -> r c w"
                            ),
                        )
                    # x0: rows r0..r0+127 (sync queue: keep the ACT SEQ
                    # free of HWDGE-serialized dma_starts so cvts run early)
                    nc.sync.dma_start(
                        x0f[:, c0:c1, ta:tb],
                        x[c0:c1, r0 : r0 + P, w0:w1].rearrange("c r w -> r c w"),
                    )
                    # xp: rows r0+1..r0+128
                    if t == NT - 1:
                        nc.sync.dma_start(
                            xpf[0 : P - 1, c0:c1, ta:tb],
                            x[c0:c1, r0 + 1 : r0 + P, w0:w1].rearrange(
                                "c r w -> r c w"
                            ),
                        )
                    else:
                        nc.sync.dma_start(
                            xpf[:, c0:c1, ta:tb],
                            x[c0:c1, r0 + 1 : r0 + P + 1, w0:w1].rearrange(
                                "c r w -> r c w"
                            ),
                        )
                    # bf16 conversions on ACT (full partition range; pad cols
                    # handled by the bf16 memsets above). Exception: the very
                    # first piece converts on the idle DVE -- it skips the
                    # ACT->DVE semaphore hop that gates the pipeline fill.
                    cvt = (
                        nc.vector.tensor_copy
                        if (t == 0 and (c0, c1, w0, w1) == chunks[0])
                        else nc.scalar.copy
                    )
                    cvt(xm[:, c0:c1, ta:tb], xmf[:, c0:c1, ta:tb])
                    cvt(x0[:, c0:c1, ta:tb], x0f[:, c0:c1, ta:tb])
                    cvt(xp[:, c0:c1, ta:tb], xpf[:, c0:c1, ta:tb])

            def emit_compute(t, chunks, store_chunks=None, fuse_last=False):
                r0 = t * P
                d = band[t]
                xm, x0, xp = d["xm"], d["x0"], d["xp"]
                m1 = pool.tile([P, C, SP], BF16, tag="m1")
                M1 = pool.tile([P, C, SP], BF16, tag="M1")
                lo = pool.tile([P, C, SP], BF16, tag="lo")
                t2 = pool.tile([P, C, SP], BF16, tag="t2")
                outf = opool.tile([P, C, W], F32, tag="outf")
                if fuse_last:
                    outfB = opool.tile([P, C, W - TB], F32, tag="outfB", name="outfB")
                else:
                    outfB = None
                for c0, c1, w0, w1 in chunks:
                    # stage V over tile cols [w0, w1+2); adjacent chunks
                    # recompute the 2 overlap cols (same values, benign)
                    va, vb = w0, w1 + 2
                    tt(m1[:, c0:c1, va:vb], xm[:, c0:c1, va:vb], x0[:, c0:c1, va:vb], MIN)
                    tt(M1[:, c0:c1, va:vb], xm[:, c0:c1, va:vb], x0[:, c0:c1, va:vb], MAX)
                    tt(lo[:, c0:c1, va:vb], m1[:, c0:c1, va:vb], xp[:, c0:c1, va:vb], MIN)
                    tt(t2[:, c0:c1, va:vb], M1[:, c0:c1, va:vb], xp[:, c0:c1, va:vb], MIN)
                    tt(m1[:, c0:c1, va:vb], m1[:, c0:c1, va:vb], t2[:, c0:c1, va:vb], MAX)  # me
                    tt(M1[:, c0:c1, va:vb], M1[:, c0:c1, va:vb], xp[:, c0:c1, va:vb], MAX)  # hi
                    me, hi = m1, M1
                    # stage H over out cols [w0, w1); reuse dead tiles:
                    # pa->xm, pc->xp, q->x0, p->t2 (names = storage)
                    # partials confined to [w0, w1): the w1'th value is never
                    # used, and writing it would clobber the aliased x tiles
                    # at the next col-chunk's V halo
                    h1 = w1 + 1
                    tt(xm[:, c0:c1, w0:w1], lo[:, c0:c1, w0:w1], lo[:, c0:c1, w0 + 1 : h1], MAX)  # pa
                    tt(xp[:, c0:c1, w0:w1], hi[:, c0:c1, w0:w1], hi[:, c0:c1, w0 + 1 : h1], MIN)  # pc
                    tt(x0[:, c0:c1, w0:w1], me[:, c0:c1, w0:w1], me[:, c0:c1, w0 + 1 : h1], MAX)  # q
                    tt(t2[:, c0:c1, w0:w1], me[:, c0:c1, w0:w1], me[:, c0:c1, w0 + 1 : h1], MIN)  # p
                    tt(xm[:, c0:c1, w0:w1], xm[:, c0:c1, w0:w1], lo[:, c0:c1, w0 + 2 : vb], MAX)  # A
                    tt(xp[:, c0:c1, w0:w1], xp[:, c0:c1, w0:w1], hi[:, c0:c1, w0 + 2 : vb], MIN)  # Cc
                    tt(x0[:, c0:c1, w0:w1], x0[:, c0:c1, w0:w1], me[:, c0:c1, w0 + 2 : vb], MIN)  # b1
                    tt(x0[:, c0:c1, w0:w1], t2[:, c0:c1, w0:w1], x0[:, c0:c1, w0:w1], MAX)  # B
                    A, B, Cc = xm, x0, xp
                    tt(lo[:, c0:c1, w0:w1], A[:, c0:c1, w0:w1], B[:, c0:c1, w0:w1], MIN)  # m2
                    tt(xm[:, c0:c1, w0:w1], A[:, c0:c1, w0:w1], B[:, c0:c1, w0:w1], MAX)  # M2
                    tt(xp[:, c0:c1, w0:w1], xm[:, c0:c1, w0:w1], Cc[:, c0:c1, w0:w1], MIN)  # t3
                    if fuse_last and (c0, c1, w0, w1) == chunks[-1]:
                        # final piece: write f32 directly (fp32 rate on this
                        # one op) into its own tile so earlier stores don't
                        # pick up a false dep on the last DVE op
                        tt(outfB[:, c0:c1, 0 : w1 - w0], xp[:, c0:c1, w0:w1], lo[:, c0:c1, w0:w1], MAX)
                    else:
                        tt(lo[:, c0:c1, w0:w1], xp[:, c0:c1, w0:w1], lo[:, c0:c1, w0:w1], MAX)  # out
                # output conversion + store, split for tail overlap
                if len(chunks) == 1:
                    oc = ((0, 2, 0, W), (2, 3, 0, W))
                elif store_chunks is not None:
                    oc = store_chunks
                else:
                    oc = chunks
                for ch in oc:
                    c0, c1, w0, w1 = ch[:4]
                    q = ch[4] if len(ch) > 4 else "a"
                    ce = ch[5] if len(ch) > 5 else "a"
                    if not (fuse_last and ch == oc[-1]):
                        cvt = nc.scalar.copy if ce == "a" else nc.vector.tensor_copy
                        cvt(outf[:, c0:c1, w0:w1], lo[:, c0:c1, w0:w1])
                    eng = {"a": nc.scalar, "s": nc.sync, "p": nc.gpsimd}[q]
                    if fuse_last and ch == oc[-1]:
                        src = outfB[:, c0:c1, 0 : w1 - w0]
                    else:
                        src = outf[:, c0:c1, w0:w1]
                    eng.dma_start(
                        y[c0:c1, r0 : r0 + P, w0:w1].rearrange("c r w -> r c w"),
                        src,
                    )

            full = [(0, C, 0, W)]
            # band 0: small first piece so DVE starts early (load split 2 cols
            # past the compute split so piece 1's V halo stays in load 1)
            load0 = [(0, 1, 0, s0), (0, 1, s0, W), (1, 3, 0, W)]
            comp0 = [(0, 1, 0, s0 - 2), (0, 1, s0 - 2, W), (1, 3, 0, W)]
            # last band: small last piece for a short drain; stores fan out
            # across the ACT and SP queues so the drain chains run in parallel
            TB = 928  # tail split col
            tailc = [(0, 3, 0, TB), (0, 3, TB, W)]
            tail_store = [
                (0, 2, 0, TB, "a", "a"),
                (2, 3, 0, TB, "s", "v"),
                (0, 3, TB, W, "p", "a"),
            ]
            emit_load(0, load0)
            emit_load(1, full)
            for t in range(NT):
                if t == 0:
                    chunks = comp0
                elif t == NT - 1:
                    chunks = tailc
                else:
                    chunks = full
                emit_compute(t, chunks,
                             tail_store if t == NT - 1 else None,
                             fuse_last=(t == NT - 1))
                if t + 2 < NT:
                    emit_load(t + 2, full)

    _legalize_waits(nc)
    return nc


_NC = None


def kernel(input):
    global _NC
    if _NC is None:
        _NC = build()
    input = np.asarray(input, dtype=np.float32)
    in_maps = [{"x": np.ascontiguousarray(input[i])} for i in range(input.shape[0])]
    res = run_bass_kernel_spmd(_NC, in_maps, core_ids=list(range(len(in_maps))))
    return np.stack([r["y"] for r in res.results], axis=0)



# revision 21
# speedup vs baseline: 1.0063x; 1.0018x over previous
"""3x3 MedianBlur (zero-padded) for (8, 3, 1024, 1024) fp32 on 8 trn2 NeuronCores.

v2: bf16 compute path. The DVE runs tensor_tensor at 2x for packed 2-byte
dtypes (0.52 ns/elem vs 1.04 for fp32), and bf16 keeps the median exact to
~2^-8 relative (selection network only -- no arithmetic), far inside the 2e-2
gate. bf16 denormal range starts at 1e-38 so randn values never flush (fp16
would flush |x|<6e-5 and blow the max-rel-err metric).

  - Pure data parallel: batch element i -> core i.
  - Per core: 8 row-bands of 128 rows; rows live in partitions, (channel, col)
    in the free dim ([128, 3, 1026] tiles). Vertical window alignment comes
    free from DMA: each band is loaded 3x from HBM at row offsets -1/0/+1
    (fp32), then the ACT engine converts each to bf16 (ACT is otherwise idle;
    DVE is the bottleneck). Pool does the zero-pad memsets; PE idle.
  - Exact separable median-of-9: 18 bf16 min/max tensor_tensor ops per band
    on the DVE, all full-width, with in-place col-aligned tile reuse:
      stage V (6): m1,M1 = minmax(xm,x0); lo = min(m1,xp); t2 = min(M1,xp);
                   me = max(m1,t2) [->m1]; hi = max(M1,xp) [->M1]
      stage H (12): A = max3(lo), C = min3(hi), B = med3(me) via pair
                   partials, out = med3(A,B,C)
  - ACT converts the final bf16 band back to fp32 (split by channel so the
    store overlaps the next band's compute); stores on the scalar queue.
  - Software pipelining by emission order: band t+1's loads+converts are
    emitted before band t's compute so the in-order ACT stream never parks
    conversions behind an output conversion. Band 0 is emitted per-channel
    to cut the pipeline fill; the last band is split at col 832 so the final
    store chain drains behind a small compute piece.
  - All loads ride the sync queue (scalar-queue dma_starts hold the ACT SEQ
    through the shared HWDGE generator and delay conversions); band 0's first
    channel is col-halved so the DVE starts ~5.5us in; the last band splits
    at col 832 with per-channel stores, and its final piece's last op writes
    f32 directly (one op at fp32 rate buys the drain an ACT-conversion hop).
  - Band 0's first piece converts on the (then-idle) DVE, skipping the
    ACT->DVE semaphore hop in the fill chain.
  - f32 staging tiles live in a single-buffered pool (dead after the
    bf16 conversion; single-buffering trims semaphore traffic and SBUF).
  - Cost-model makespan: 250.0 us (DVE busy ~243.7 us / 97.4%; fp32 DVE
    floor for this 18-op network is ~470 us -- the bf16 2x mode is the win.
    Pool cannot run tensor_tensor in this walrus build (codegen emits a
    NEFF that fails to load), ACT has no elementwise 2-tensor op, and PE is
    linear-only, so the DVE does all 18 ops).
  - Tail drain: the last band splits at col 928; its three channel stores
    fan out over the ACT, SP and Pool DMA queues (ch2's f32 conversion runs
    on the DVE so each queue's conv->store chain is independent), and the
    final 96-col piece writes f32 directly into its own tile (outfB) so
    earlier stores see no false dep on the last DVE op. Output conversions
    live in a bufs=3 pool so band t's conversion never waits on band t-1's
    store DMA completing (DMA sem prop is 900ns). Post-DVE drain ~3.0us,
    at the structural floor (DMA start latency + sem prop + barrier).
  - Band 0's first piece covers cols 0..518 (sweep-tuned): zero mid-stream
    DVE gaps; makespan = fill (~3.4us) + DVE busy + drain.

The walrus build accepts at most 1 inline sync wait per instruction
(2 on EventSemaphore); Tile emits more, so _legalize_waits() spills excess
waits onto same-engine NoOps placed immediately before the instruction.
"""
import sys

sys.path.insert(0, "/opt/trn_rl_repo")

import numpy as np

import concourse.bass as bass
import concourse.mybir as mybir
from concourse.bass_utils import run_bass_kernel_spmd
from concourse.tile import TileContext


C, H, W = 3, 1024, 1024
P = 128
NT = H // P
SP = W + 2      # padded width: tile col c <-> DRAM col c-1
F32 = mybir.dt.float32
BF16 = mybir.dt.bfloat16
MIN = mybir.AluOpType.min
MAX = mybir.AluOpType.max


def _legalize_waits(nc):
    """Split sync_info.on_wait lists that exceed this walrus's per-instruction
    capacity (1; 2 for EventSemaphore) onto preceding same-engine NoOps."""
    for f in nc.m.functions:
        for bb in f.blocks:
            new_insts = []
            for ins in bb.instructions:
                si = ins.sync_info
                cap = 2 if ins.opcode == "EventSemaphore" else 1
                if si is not None and len(si.on_wait) > cap:
                    waits = list(si.on_wait)
                    extra, keep = waits[:-cap], waits[-cap:]
                    for w in extra:
                        nop = mybir.InstNoOp(
                            name=nc.get_next_instruction_name(),
                            ins=[],
                            outs=[],
                            engine=ins.engine,
                        )
                        nop.sync_info = mybir.SyncInfo(on_wait=[w], on_update=[])
                        new_insts.append(nop)
                    ins.sync_info = mybir.SyncInfo(
                        on_wait=keep, on_update=list(si.on_update)
                    )
                new_insts.append(ins)
            bb.instructions = new_insts


def build(bufs=2, s0=520):
    nc = bass.Bass()
    x = nc.dram_tensor("x", [C, H, W], F32, kind="ExternalInput")
    y = nc.dram_tensor("y", [C, H, W], F32, kind="ExternalOutput")
    tt = nc.vector.tensor_tensor

    with TileContext(nc) as tc:
        with (
            tc.tile_pool(name="pool", bufs=bufs) as pool,
            tc.tile_pool(name="fpool", bufs=1) as fpool,
            tc.tile_pool(name="opool", bufs=3) as opool,
        ):
            band = {}  # t -> dict of tiles

            def emit_load(t, chunks):
                """Loads (f32) + bf16 conversion + pad memsets for band t.

                chunks: list of (c0, c1, w0, w1) channel/DRAM-col pieces."""
                r0 = t * P
                xmf = fpool.tile([P, C, SP], F32, tag="xmf")
                x0f = fpool.tile([P, C, SP], F32, tag="x0f")
                xpf = fpool.tile([P, C, SP], F32, tag="xpf")
                xm = pool.tile([P, C, SP], BF16, tag="xm")
                x0 = pool.tile([P, C, SP], BF16, tag="x0")
                xp = pool.tile([P, C, SP], BF16, tag="xp")
                band[t] = {"xm": xm, "x0": x0, "xp": xp}
                # zero pad: tile cols 0 and 1025 of each bf16 tile
                for z in (xm, x0, xp):
                    e = z[:].copy()
                    e.ap = e.ap.__class__(
                        [tuple(e.ap[0]), (SP, C), (W + 1, 2)]
                    )
                    nc.gpsimd.memset(e, 0.0)
                for c0, c1, w0, w1 in chunks:
                    ta, tb = w0 + 1, w1 + 1  # tile col range of this piece
                    # pad rows: memset the f32 staging pad row before
                    # conversion (compute APs must start at partition 0, so
                    # [0:1] is legal but [1:P] is not; conversions cover [0:P])
                    if t == 0:
                        nc.gpsimd.memset(xmf[0:1, c0:c1, ta:tb], 0.0)
                    if t == NT - 1:
                        # base must be 32-aligned; DMA rewrites rows 96..126
                        nc.gpsimd.memset(xpf[96:P, c0:c1, ta:tb], 0.0)
                    # xm first: V's first two ops need only xm+x0, so their
                    # conversions gate DVE start. xm: rows r0-1..r0+126
                    if t == 0:
                        nc.sync.dma_start(
                            xmf[1:P, c0:c1, ta:tb],
                            x[c0:c1, 0 : P - 1, w0:w1].rearrange("c r w -> r c w"),
                        )
                    else:
                        nc.sync.dma_start(
                            xmf[:, c0:c1, ta:tb],
                            x[c0:c1, r0 - 1 : r0 + P - 1, w0:w1].rearrange(
                                "c r w -> r c w"
                            ),
                        )
                    # x0: rows r0..r0+127 (sync queue: keep the ACT SEQ
                    # free of HWDGE-serialized dma_starts so cvts run early)
                    nc.sync.dma_start(
                        x0f[:, c0:c1, ta:tb],
                        x[c0:c1, r0 : r0 + P, w0:w1].rearrange("c r w -> r c w"),
                    )
                    # xp: rows r0+1..r0+128
                    if t == NT - 1:
                        nc.sync.dma_start(
                            xpf[0 : P - 1, c0:c1, ta:tb],
                            x[c0:c1, r0 + 1 : r0 + P, w0:w1].rearrange(
                                "c r w -> r c w"
                            ),
                        )
                    else:
                        nc.sync.dma_start(
                            xpf[:, c0:c1, ta:tb],
                            x[c0:c1, r0 + 1 : r0 + P + 1, w0:w1].rearrange(
                                "c r w -> r c w"
                            ),
                        )
                    # bf16 conversions on ACT (full partition range; pad cols
                    # handled by the bf16 memsets above). Exception: the very
                    # first piece converts on the idle DVE -- it skips the
                    # ACT->DVE semaphore hop that gates the pipeline fill.
                    cvt = (
                        nc.vector.tensor_copy
                        if (t == 0 and (c0, c1, w0, w1) == chunks[0])
                        else nc.scalar.copy
                    )
                    cvt(xm[:, c0:c1, ta:tb], xmf[:, c0:c1, ta:tb])
                    cvt(x0[:, c0:c1, ta:tb], x0f[:, c0:c1, ta:tb])
                    cvt(xp[:, c0:c1, ta:tb], xpf[:, c0:c1, ta:tb])

            def emit_compute(t, chunks, store_chunks=None, fuse_last=False):
                r0 = t * P
                d = band[t]
                xm, x0, xp = d["xm"], d["x0"], d["xp"]
                m1 = pool.tile([P, C, SP], BF16, tag="m1")
                M1 = pool.tile([P, C, SP], BF16, tag="M1")
                lo = pool.tile([P, C, SP], BF16, tag="lo")
                t2 = pool.tile([P, C, SP], BF16, tag="t2")
                outf = opool.tile([P, C, W], F32, tag="outf")
                if fuse_last:
                    outfB = opool.tile([P, C, W - TB], F32, tag="outfB", name="outfB")
                else:
                    outfB = None
                for c0, c1, w0, w1 in chunks:
                    # stage V over tile cols [w0, w1+2); adjacent chunks
                    # recompute the 2 overlap cols (same values, benign)
                    va, vb = w0, w1 + 2
                    tt(m1[:, c0:c1, va:vb], xm[:, c0:c1, va:vb], x0[:, c0:c1, va:vb], MIN)
                    tt(M1[:, c0:c1, va:vb], xm[:, c0:c1, va:vb], x0[:, c0:c1, va:vb], MAX)
                    tt(lo[:, c0:c1, va:vb], m1[:, c0:c1, va:vb], xp[:, c0:c1, va:vb], MIN)
                    tt(t2[:, c0:c1, va:vb], M1[:, c0:c1, va:vb], xp[:, c0:c1, va:vb], MIN)
                    tt(m1[:, c0:c1, va:vb], m1[:, c0:c1, va:vb], t2[:, c0:c1, va:vb], MAX)  # me
                    tt(M1[:, c0:c1, va:vb], M1[:, c0:c1, va:vb], xp[:, c0:c1, va:vb], MAX)  # hi
                    me, hi = m1, M1
                    # stage H over out cols [w0, w1); reuse dead tiles:
                    # pa->xm, pc->xp, q->x0, p->t2 (names = storage)
                    # partials confined to [w0, w1): the w1'th value is never
                    # used, and writing it would clobber the aliased x tiles
                    # at the next col-chunk's V halo
                    h1 = w1 + 1
                    tt(xm[:, c0:c1, w0:w1], lo[:, c0:c1, w0:w1], lo[:, c0:c1, w0 + 1 : h1], MAX)  # pa
                    tt(xp[:, c0:c1, w0:w1], hi[:, c0:c1, w0:w1], hi[:, c0:c1, w0 + 1 : h1], MIN)  # pc
                    tt(x0[:, c0:c1, w0:w1], me[:, c0:c1, w0:w1], me[:, c0:c1, w0 + 1 : h1], MAX)  # q
                    tt(t2[:, c0:c1, w0:w1], me[:, c0:c1, w0:w1], me[:, c0:c1, w0 + 1 : h1], MIN)  # p
                    tt(xm[:, c0:c1, w0:w1], xm[:, c0:c1, w0:w1], lo[:, c0:c1, w0 + 2 : vb], MAX)  # A
                    tt(xp[:, c0:c1, w0:w1], xp[:, c0:c1, w0:w1], hi[:, c0:c1, w0 + 2 : vb], MIN)  # Cc
                    tt(x0[:, c0:c1, w0:w1], x0[:, c0:c1, w0:w1], me[:, c0:c1, w0 + 2 : vb], MIN)  # b1
                    tt(x0[:, c0:c1, w0:w1], t2[:, c0:c1, w0:w1], x0[:, c0:c1, w0:w1], MAX)  # B
                    A, B, Cc = xm, x0, xp
                    tt(lo[:, c0:c1, w0:w1], A[:, c0:c1, w0:w1], B[:, c0:c1, w0:w1], MIN)  # m2
                    tt(xm[:, c0:c1, w0:w1], A[:, c0:c1, w0:w1], B[:, c0:c1, w0:w1], MAX)  # M2
                    tt(xp[:, c0:c1, w0:w1], xm[:, c0:c1, w0:w1], Cc[:, c0:c1, w0:w1], MIN)  # t3
                    if fuse_last and (c0, c1, w0, w1) == chunks[-1]:
                        # final piece: write f32 directly (fp32 rate on this
                        # one op) into its own tile so earlier stores don't
                        # pick up a false dep on the last DVE op
                        tt(outfB[:, c0:c1, 0 : w1 - w0], xp[:, c0:c1, w0:w1], lo[:, c0:c1, w0:w1], MAX)
                    else:
                        tt(lo[:, c0:c1, w0:w1], xp[:, c0:c1, w0:w1], lo[:, c0:c1, w0:w1], MAX)  # out
                # output conversion + store, split for tail overlap
                if len(chunks) == 1:
                    oc = ((0, 2, 0, W), (2, 3, 0, W))
                elif store_chunks is not None:
                    oc = store_chunks
                else:
                    oc = chunks
                for ch in oc:
                    c0, c1, w0, w1 = ch[:4]
                    q = ch[4] if len(ch) > 4 else "a"
                    ce = ch[5] if len(ch) > 5 else "a"
                    if not (fuse_last and ch == oc[-1]):
                        cvt = nc.scalar.copy if ce == "a" else nc.vector.tensor_copy
                        cvt(outf[:, c0:c1, w0:w1], lo[:, c0:c1, w0:w1])
                    eng = {"a": nc.scalar, "s": nc.sync, "p": nc.gpsimd}[q]
                    if fuse_last and ch == oc[-1]:
                        src = outfB[:, c0:c1, 0 : w1 - w0]
                    else:
                        src = outf[:, c0:c1, w0:w1]
                    eng.dma_start(
                        y[c0:c1, r0 : r0 + P, w0:w1].rearrange("c r w -> r c w"),
                        src,
                    )

            full = [(0, C, 0, W)]
            # band 0: small first piece so DVE starts early (load split 2 cols
            # past the compute split so piece 1's V halo stays in load 1)
            load0 = [(0, 1, 0, s0), (0, 1, s0, W), (1, 3, 0, W)]
            comp0 = [(0, 1, 0, s0 - 2), (0, 1, s0 - 2, W), (1, 3, 0, W)]
            # last band: small last piece for a short drain; stores fan out
            # across the ACT and SP queues so the drain chains run in parallel
            TB = 928  # tail split col
            tailc = [(0, 3, 0, TB), (0, 3, TB, W)]
            tail_store = [
                (0, 1, 0, TB, "a", "a"),
                (1, 2, 0, TB, "s", "a"),
                (2, 3, 0, TB, "p", "v"),
                (0, 3, TB, W, "a", "a"),
            ]
            emit_load(0, load0)
            emit_load(1, full)
            for t in range(NT):
                if t == 0:
                    chunks = comp0
                elif t == NT - 1:
                    chunks = tailc
                else:
                    chunks = full
                emit_compute(t, chunks,
                             tail_store if t == NT - 1 else None,
                             fuse_last=(t == NT - 1))
                if t + 2 < NT:
                    emit_load(t + 2, full)

    _legalize_waits(nc)
    return nc


_NC = None


def kernel(input):
    global _NC
    if _NC is None:
        _NC = build()
    input = np.asarray(input, dtype=np.float32)
    in_maps = [{"x": np.ascontiguousarray(input[i])} for i in range(input.shape[0])]
    res = run_bass_kernel_spmd(_NC, in_maps, core_ids=list(range(len(in_maps))))
    return np.stack([r["y"] for r in res.results], axis=0)

